# revision 27
# baseline (speedup 1.0000x reference)
"""Block-causal (block=64) MHA + qkv/out projections on 8 NeuronCores.

Sharding: 8 cores = 2 batches x 4 head-groups (4 heads each).
Per core: qkv projection for its heads, block-causal attention for 4 heads
(processed as 2 head-pairs packed across the 128 partitions), partial output
projection over its 256 channels. Host sums the 4 partials per batch + bias.

On-chip layout is feature-major (transposed): scores are computed transposed
(S^T[k, q] = k . q) so no on-chip transposes are needed anywhere; softmax
denominators (sums over the key/partition axis) come from an all-ones matmul
on the PE, broadcast across 64 partitions. exp runs on ScalarE straight out
of PSUM. The diagonal 128-key tiles are split into two 64-key sub-blocks with
N-restricted matmuls, so block-causality costs no masking ops.
"""

import os

import numpy as np

import concourse.bass as bass
import concourse.tile as tile
from concourse import bacc
from concourse import mybir

B, N, C = 2, 2048, 1024
H, HD = 16, 64
HPC = 4  # heads per core
CSL = HPC * HD  # 256 channel slice per core
QKW = 2 * CSL  # 512: q then k output channels
NCORES = 8
QBLK = 512
NQB = N // QBLK  # 4
NT = N // 128  # 16 seq tiles of 128
SCALE = HD**-0.5
F32 = mybir.dt.float32
BF16 = mybir.dt.bfloat16


def _mm(ap):
    """Matmul operand view (operands are natively bf16 now)."""
    return ap


def build_nc():
    nc = bacc.Bacc("TRN2", target_bir_lowering=False, debug=False, num_devices=NCORES)

    xT_d = nc.dram_tensor("xT", [8, 128, N], BF16, kind="ExternalInput")
    wqk_d = nc.dram_tensor("wqkT", [8, 128, QKW], BF16, kind="ExternalInput")
    wv_d = nc.dram_tensor("wvT", [8, 128, CSL], BF16, kind="ExternalInput")
    wp_d = nc.dram_tensor("wpT", [2, 128, C], BF16, kind="ExternalInput")
    y_d = nc.dram_tensor("y", [N, C], BF16, kind="ExternalOutput")

    with tile.TileContext(nc) as tc:
        with (
            tc.tile_pool(name="persist", bufs=1) as persist,
            tc.tile_pool(name="pt", bufs=2) as pt_pool,
            tc.tile_pool(name="rc", bufs=2) as rc_pool,
            tc.tile_pool(name="yout", bufs=3) as y_pool,
            tc.tile_pool(name="psmm", bufs=2, space="PSUM") as ps_mm,
            tc.tile_pool(name="pssc", bufs=2, space="PSUM") as ps_sc,
            tc.tile_pool(name="psacc", bufs=1, space="PSUM") as ps_acc,
        ):
            # ---- load inputs ----
            # consolidated tiles, few big DMAs: each DMA_DIRECT2D costs
            # ~650 ns of Sync-queue issue time, so 26 small loads would delay
            # the first matmul by ~17 us. Order: wqk first (phase 1 needs it),
            # x in 512-token chunks so the first projection group starts
            # after ~2 MB instead of 6.3 MB.
            x_all = persist.tile([128, 8 * N], BF16, tag="xall", name="xall")
            wqk_all = persist.tile([128, 8 * QKW], BF16, tag="wqkall", name="wqkall")
            wv_all = persist.tile([128, 8 * CSL], BF16, tag="wvall", name="wvall")
            xts = [x_all[:, i * N : (i + 1) * N] for i in range(8)]
            wqks = [wqk_all[:, i * QKW : (i + 1) * QKW] for i in range(8)]
            wvs = [wv_all[:, i * CSL : (i + 1) * CSL] for i in range(8)]
            wps = [persist.tile([128, C], BF16, tag=f"wp{i}", name=f"wp{i}") for i in range(2)]
            nc.sync.dma_start(
                out=wqk_all.rearrange("p (c w) -> p c w", c=8),
                in_=wqk_d[:, :, :].rearrange("c p w -> p c w"),
            )
            for nb in range(NQB):
                sl = slice(nb * QBLK, (nb + 1) * QBLK)
                nc.sync.dma_start(
                    out=x_all.rearrange("p (c n) -> p c n", c=8)[:, :, sl],
                    in_=xT_d[:, :, sl].rearrange("c p n -> p c n"),
                )
            nc.sync.dma_start(
                out=wv_all.rearrange("p (c w) -> p c w", c=8),
                in_=wv_d[:, :, :].rearrange("c p w -> p c w"),
            )
            for pr in range(2):
                nc.sync.dma_start(out=wps[pr], in_=wp_d[pr])

            ones_t = persist.tile([128, 128], BF16, tag="ones")
            nc.vector.memset(ones_t, 1.0)

            # ---- phase 1: q/k projection, transposed outputs ----
            # qkT tiles: 0 = q heads(0,1), 1 = q heads(2,3), 2 = k(0,1), 3 = k(3,4)
            # within a tile: partitions 0:64 = even head dims, 64:128 = odd head.
            qkT = [persist.tile([128, N], BF16, tag=f"qk{t}", name=f"qk{t}") for t in range(4)]
            for dt_ in range(4):
                for nb in range(NQB):
                    ps = ps_mm.tile([128, QBLK], F32, tag="mm")
                    for ct in range(8):
                        nc.tensor.matmul(
                            ps,
                            lhsT=_mm(wqks[ct][:, dt_ * 128 : (dt_ + 1) * 128]),
                            rhs=_mm(xts[ct][:, nb * QBLK : (nb + 1) * QBLK]),
                            start=(ct == 0),
                            stop=(ct == 7),
                        )
                    nc.vector.tensor_copy(
                        out=qkT[dt_][:, nb * QBLK : (nb + 1) * QBLK], in_=ps
                    )

            # ---- phase 2: v projection, natural layout [n, 4*(64+1)] ----
            # per head: [v(64) | 1] so the PV matmul's 65-col stationary
            # computes PV rows 0:64 AND the softmax denominator in row 64 of
            # the same accumulation group (kills the separate ones-matmuls).
            VW = 4 * 65  # 260
            v_sb = [persist.tile([128, VW], BF16, tag=f"v{t}", name=f"v{t}") for t in range(NT)]
            for nt in range(NT):
                nc.vector.memset(
                    v_sb[nt].rearrange("p (h e) -> p h e", h=4)[:, :, 64:65], 1.0
                )
                ps = ps_mm.tile([128, CSL], F32, tag="mm")
                for ct in range(8):
                    nc.tensor.matmul(
                        ps,
                        lhsT=_mm(xts[ct][:, nt * 128 : (nt + 1) * 128]),
                        rhs=_mm(wvs[ct]),
                        start=(ct == 0),
                        stop=(ct == 7),
                    )
                nc.vector.tensor_copy(
                    out=v_sb[nt].rearrange("p (h e) -> p h e", h=4)[:, :, 0:64],
                    in_=ps.rearrange("p (h d) -> p h d", h=4),
                )

            # ---- phase 3+4: attention (per 512-query block), then out-proj ----
            PHASES = int(os.environ.get("KERNEL_PHASES", "3"))
            attnT = [persist.tile([128, N], BF16, tag=f"at{p}", name=f"at{p}") for p in range(2)]
            if PHASES == 1:
                for p in range(2):
                    nc.vector.memset(attnT[p], 0.0)
            def emit_outproj(qi):
                # out-proj for query block qi's 4 row tiles; emitted DELAYED —
                # interleaved after the next block's first attention tile — so
                # the PE has queued work while the normalize chain finishes.
                for nt in range(4 * qi, 4 * qi + 4):
                    ysb = y_pool.tile([128, C], BF16, tag="y")
                    for cb in range(2):
                        psy = ps_mm.tile([128, QBLK], F32, tag="mm")
                        for pr in range(2):
                            nc.tensor.matmul(
                                psy,
                                lhsT=_mm(attnT[pr][:, nt * 128 : (nt + 1) * 128]),
                                rhs=_mm(wps[pr][:, cb * QBLK : (cb + 1) * QBLK]),
                                start=(pr == 0),
                                stop=(pr == 1),
                            )
                        nc.vector.tensor_copy(
                            out=ysb[:, cb * QBLK : (cb + 1) * QBLK], in_=psy
                        )
                    nc.sync.dma_start(out=y_d[nt * 128 : (nt + 1) * 128, :], in_=ysb)

            pending_outproj = [None]

            def flush_outproj():
                if pending_outproj[0] is not None:
                    emit_outproj(pending_outproj[0])
                    pending_outproj[0] = None

            for qi in range(NQB if PHASES >= 2 else 0):
                for pair in range(2):
                    qt = qkT[pair]
                    kt_t = qkT[2 + pair]
                    qs = slice(qi * QBLK, (qi + 1) * QBLK)

                    # one PSUM bank per head: PV rows 0:64 + softmax-sum row 64
                    # (both accumulation groups base-partition-0 — banks can't
                    # mix base-0 and base-64 groups).
                    at_bA = ps_acc.tile([128, QBLK], F32, tag="atA", name="at_bA")
                    at_bB = ps_acc.tile([128, QBLK], F32, tag="atB", name="at_bB")
                    vcA = pair * 130  # lhsT col base: [vA|1]
                    vcB = pair * 130 + 65  # [vB|1]

                    n_reg = 4 * qi
                    ATT_RECT = os.environ.get("ATT_RECT", "0") == "1"
                    diag_upto = int(os.environ.get("ATT_DIAG_UPTO", "8"))
                    if qi * 2 + pair >= diag_upto:
                        ATT_RECT = True
                    if ATT_RECT:
                        n_reg = 4 * qi + 4  # probe: no diagonal handling at all
                    # per-bank accumulation-group flags: each head brackets
                    # its own group of fused PV+sum matmuls.
                    n_per_range = n_reg + (0 if ATT_RECT else 4)
                    at_A, at_B = [0], [0]

                    def fl(cnt, total=n_per_range):
                        i = cnt[0]
                        cnt[0] += 1
                        return dict(start=(i == 0), stop=(i == total - 1))

                    # fully-causal key tiles: whole [128k x 512q] rectangles
                    for kt in range(n_reg):
                        ks = slice(kt * 128, (kt + 1) * 128)
                        psA = ps_sc.tile([128, QBLK], F32, tag="sA")
                        psB = ps_sc.tile([128, QBLK], F32, tag="sB")
                        nc.tensor.matmul(
                            psA, lhsT=_mm(kt_t[0:64, ks]), rhs=_mm(qt[0:64, qs]),
                            start=True, stop=True,
                        )
                        nc.tensor.matmul(
                            psB, lhsT=_mm(kt_t[64:128, ks]), rhs=_mm(qt[64:128, qs]),
                            start=True, stop=True,
                        )
                        pA = pt_pool.tile([128, QBLK], BF16, tag="pA")
                        pB = pt_pool.tile([128, QBLK], BF16, tag="pB")
                        nc.scalar.activation(
                            out=pA, in_=psA, func=mybir.ActivationFunctionType.Exp,
                            scale=SCALE,
                        )
                        nc.scalar.activation(
                            out=pB, in_=psB, func=mybir.ActivationFunctionType.Exp,
                            scale=SCALE,
                        )
                        nc.tensor.matmul(
                            at_bA[0:65, :], lhsT=_mm(v_sb[kt][:, vcA : vcA + 65]),
                            rhs=_mm(pA), **fl(at_A),
                        )
                        nc.tensor.matmul(
                            at_bB[0:65, :], lhsT=_mm(v_sb[kt][:, vcB : vcB + 65]),
                            rhs=_mm(pB), **fl(at_B),
                        )
                        if kt == 0:
                            flush_outproj()

                    # diagonal key tiles: two 64-key sub-blocks, N-restricted
                    for j in ([] if ATT_RECT else range(4)):
                        kt = 4 * qi + j
                        q0 = 128 * j  # first allowed q offset for keys [0,64)
                        q1 = 128 * j + 64  # for keys [64,128)
                        if os.environ.get("ATT_DIAG_FULLN", "0") == "1":
                            q0 = q1 = 0  # probe: quadrant MMs, full N
                        k0 = slice(kt * 128, kt * 128 + 64)
                        k1 = slice(kt * 128 + 64, (kt + 1) * 128)
                        psA = ps_sc.tile([128, QBLK], F32, tag="sA")
                        psB = ps_sc.tile([128, QBLK], F32, tag="sB")
                        qsl0 = slice(qi * QBLK + q0, (qi + 1) * QBLK)
                        pA = pt_pool.tile([128, QBLK], BF16, tag="pA")
                        pB = pt_pool.tile([128, QBLK], BF16, tag="pB")
                        for ph, ps_s, p_s in ((0, psA, pA), (64, psB, pB)):
                            hd_sl = slice(ph, ph + 64)
                            if q0 == 0:
                                # both key halves see the full query range:
                                # one M=128 matmul instead of two (saves a
                                # 512-row stream + an LDWEIGHTS)
                                nc.tensor.matmul(
                                    ps_s,
                                    lhsT=_mm(kt_t[hd_sl, kt * 128 : (kt + 1) * 128]),
                                    rhs=_mm(qt[hd_sl, qs]), start=True, stop=True,
                                )
                            else:
                                # sub1 computes from q0 (not q1) so the bank is
                                # fully written and ONE exp covers both halves —
                                # two exps would read the bank while the second
                                # sub-MM still writes it (fatal PSUM collision).
                                nc.tensor.matmul(
                                    ps_s[0:64, q0:QBLK], lhsT=_mm(kt_t[hd_sl, k0]),
                                    rhs=_mm(qt[hd_sl, qsl0]), start=True, stop=True,
                                )
                                nc.tensor.matmul(
                                    ps_s[64:128, q0:QBLK], lhsT=_mm(kt_t[hd_sl, k1]),
                                    rhs=_mm(qt[hd_sl, qsl0]), start=True, stop=True,
                                )
                            nc.scalar.activation(
                                out=p_s[:, q0:QBLK], in_=ps_s[:, q0:QBLK],
                                func=mybir.ActivationFunctionType.Exp, scale=SCALE,
                            )
                            # zero the disallowed corner (keys k1 x queries
                            # [q0,q1)) so PV/sum can run as single K=128
                            # matmuls. Two row-split accumulating MMs would
                            # drain concurrently into the same PSUM cells —
                            # a fatal collision on hardware.
                            nc.gpsimd.memset(p_s[64:128, q0:q1], 0.0)
                        for p_s, at_c, at_b, vc in (
                            (pA, at_A, at_bA, vcA),
                            (pB, at_B, at_bB, vcB),
                        ):
                            nc.tensor.matmul(
                                at_b[0:65, q0:QBLK],
                                lhsT=_mm(v_sb[kt][:, vc : vc + 65]),
                                rhs=_mm(p_s[:, q0:QBLK]), **fl(at_c),
                            )
                        if n_reg == 0 and j == 0:
                            flush_outproj()

                    # normalize: attnT[:, qblock] = at * (1 / sm). The sums
                    # live on single partition rows (row 64 of each bank);
                    # copy them to partitions 0 / 32 so the 51-ULP approx
                    # runs 512 elems/lane on two lanes (not 1024 on one),
                    # then PE-broadcast to 64 partitions and multiply.
                    # (garbage in the untouched lanes of rows 1..63 is never
                    # read — the rcb matmuls take rows 0 and 32 only.)
                    sm_sb = rc_pool.tile([128, 2 * QBLK], F32, tag="sm")
                    nc.vector.tensor_copy(out=sm_sb[0:1, 0:QBLK], in_=at_bA[64:65, :])
                    nc.vector.tensor_copy(
                        out=sm_sb[0:1, QBLK : 2 * QBLK], in_=at_bB[64:65, :]
                    )
                    rcrow = rc_pool.tile([128, 2 * QBLK], F32, tag="rcw")
                    rc_bf = rc_pool.tile([128, 2 * QBLK], BF16, tag="rcb")
                    nc.vector.reciprocal_approx_fast(
                        out=rcrow[0:1, :], in_=sm_sb[0:1, :]
                    )
                    nc.vector.tensor_copy(out=rc_bf[0:1, :], in_=rcrow[0:1, :])
                    rcb_psA = ps_mm.tile([128, QBLK], F32, tag="mm")
                    nc.tensor.matmul(
                        rcb_psA[0:64, :], lhsT=_mm(ones_t[0:1, 0:64]),
                        rhs=_mm(rc_bf[0:1, 0:QBLK]), start=True, stop=True,
                    )
                    rcb_psB = ps_mm.tile([128, QBLK], F32, tag="mm")
                    nc.tensor.matmul(
                        rcb_psB[0:64, :], lhsT=_mm(ones_t[0:1, 0:64]),
                        rhs=_mm(rc_bf[0:1, QBLK : 2 * QBLK]), start=True, stop=True,
                    )
                    recip = rc_pool.tile([128, 2 * QBLK], F32, tag="rc")
                    nc.vector.tensor_copy(out=recip[0:64, 0:QBLK], in_=rcb_psA[0:64, :])
                    nc.vector.tensor_copy(
                        out=recip[0:64, QBLK : 2 * QBLK], in_=rcb_psB[0:64, :]
                    )
                    nc.vector.tensor_mul(
                        out=attnT[pair][0:64, qs], in0=at_bA[0:64, :],
                        in1=recip[0:64, 0:QBLK],
                    )
                    # head B: partition-shifted output (in base 0 -> out base 64)
                    nc.vector.tensor_mul(
                        out=attnT[pair][64:128, qs], in0=at_bB[0:64, :],
                        in1=recip[0:64, QBLK : 2 * QBLK],
                    )

                if PHASES >= 3:
                    pending_outproj[0] = qi
            flush_outproj()

            if PHASES < 3:
                for nt in range(NT):
                    ysb = y_pool.tile([128, C], BF16, tag="y", name="ysb_fb")
                    for cb in range(2):
                        psy = ps_mm.tile([128, QBLK], F32, tag="mm", name="psy_fb")
                        for pr in range(2):
                            nc.tensor.matmul(
                                psy,
                                lhsT=_mm(attnT[pr][:, nt * 128 : (nt + 1) * 128]),
                                rhs=_mm(wps[pr][:, cb * QBLK : (cb + 1) * QBLK]),
                                start=(pr == 0),
                                stop=(pr == 1),
                            )
                        nc.vector.tensor_copy(
                            out=ysb[:, cb * QBLK : (cb + 1) * QBLK], in_=psy
                        )
                    nc.sync.dma_start(out=y_d[nt * 128 : (nt + 1) * 128, :], in_=ysb)

    return nc


def _shard_inputs(x, w_qkv, w_proj):
    import ml_dtypes

    bf16 = np.dtype(ml_dtypes.bfloat16)
    x = np.asarray(x, dtype=np.float32).astype(bf16)
    w_qkv = np.asarray(w_qkv, dtype=np.float32).astype(bf16)
    w_proj = np.asarray(w_proj, dtype=np.float32).astype(bf16)
    xT = [np.ascontiguousarray(x[b].T).reshape(8, 128, N) for b in range(B)]
    in_maps = []
    for c in range(NCORES):
        b, g = divmod(c, 4)
        r0 = 64 * HPC * g  # 256 * g
        wq = w_qkv[r0 : r0 + CSL, :]
        wk = w_qkv[C + r0 : C + r0 + CSL, :]
        wvs = w_qkv[2 * C + r0 : 2 * C + r0 + CSL, :]
        wqkT = np.ascontiguousarray(np.concatenate([wq, wk], axis=0).T)
        wvT = np.ascontiguousarray(wvs.T)
        wpT = np.ascontiguousarray(w_proj[:, r0 : r0 + CSL].T)
        in_maps.append(
            {
                "xT": xT[b],
                "wqkT": wqkT.reshape(8, 128, QKW),
                "wvT": wvT.reshape(8, 128, CSL),
                "wpT": wpT.reshape(2, 128, C),
            }
        )
    return in_maps


def run(x, w_qkv, w_proj, b_proj, trace=False, **spmd_kwargs):
    from concourse.bass_utils import run_bass_kernel_spmd

    in_maps = _shard_inputs(x, w_qkv, w_proj)
    nc = build_nc()
    nc.finalize()
    res = run_bass_kernel_spmd(
        nc, in_maps, core_ids=list(range(NCORES)), trace=trace, **spmd_kwargs
    )
    y = np.zeros((B, N, C), np.float32)
    for c in range(NCORES):
        y[c // 4] += res.results[c]["y"].astype(np.float32)
    y += np.asarray(b_proj, dtype=np.float32)[None, None, :]
    return y, res


def kernel(x, w_qkv, w_proj, b_proj):
    y, _ = run(x, w_qkv, w_proj, b_proj, trace=False)
    return y

